# revision 1
# baseline (speedup 1.0000x reference)
"""Trainium2 Bass kernel for nn_KairosModel (2-layer TransformerConv GNN + LinkPredictor).

Self-contained: takes full (unsharded) inputs, returns the full [E, 2] output.

Strategy (edge-parallel, dst-partitioned):
  * Host relabels nodes into W windows (<=128 nodes, <=S edges each) via a
    degree-sorted snake packing; each of the 8 cores owns W/8 windows. All
    segment-softmax statistics are therefore core-local, and segment sums are
    expressed as one-hot matmuls on the PE (S = is_equal(iota, dstloc)).
  * Softmax max-subtraction is dropped (alpha is O(1) for this model), so
    a = exp(alpha)/sum(exp(alpha)); numerator and denominator are scattered
    with a single one-hot matmul and divided per node.
  * Between layers, only the small per-node projections (k2|v2 [N,200] and
    h2 [N,100]) are AllGathered; x/k/v of layer 1 are recomputed per edge.
  * LinkPredictor first layer is folded on the host:
        t1 = tanh(h_src @ A_s + h_dst @ A_d + b')   (A_* = lp_*_w @ lp1_w half)
    and the whole MLP runs feature-major so each layer's output is directly
    the next layer's stationary operand (no transposes).
"""
import math
import numpy as np

# ---------------- problem constants (hardcoded per contract) ----------------
N_FULL, E_FULL, D = 50000, 150000, 100
C = 8           # cores
NS = 128        # node slots per window
TS = 128        # edge slots per tile


# ---------------------------- host preprocessing ----------------------------

def preprocess(inputs, W, S):
    """Relabel nodes/edges into the padded window-slot space. Pure numpy."""
    src = np.asarray(inputs["edge_index"][0]).astype(np.int64)
    dst = np.asarray(inputs["edge_index"][1]).astype(np.int64)
    x = np.asarray(inputs["x"], dtype=np.float32)
    N, E = x.shape[0], src.shape[0]
    WPC = W // C
    NPAD = W * NS
    EPC = WPC * S

    deg = np.bincount(dst, minlength=N)
    order = np.argsort(-deg, kind="stable")
    k = np.arange(N) % (2 * W)
    win_of_sorted = np.where(k < W, k, 2 * W - 1 - k)
    win_of_node = np.empty(N, np.int64)
    win_of_node[order] = win_of_sorted

    # slot within window
    by_win = np.argsort(win_of_node, kind="stable")
    wcnt = np.bincount(win_of_node, minlength=W)
    wstart = np.concatenate([[0], np.cumsum(wcnt)])[:W]
    pos = np.arange(N) - wstart[win_of_node[by_win]]
    gslot = np.empty(N, np.int64)
    gslot[by_win] = win_of_node[by_win] * NS + pos
    assert wcnt.max() <= NS, wcnt.max()

    # edge slots: window w owns global edge slots [w*S, (w+1)*S)
    ewin = win_of_node[dst]
    ecnt = np.bincount(ewin, minlength=W)
    assert ecnt.max() <= S, f"bin overflow: {ecnt.max()} > {S}"
    eby = np.argsort(ewin, kind="stable")
    estart = np.concatenate([[0], np.cumsum(ecnt)])[:W]
    epos = np.arange(E) - estart[ewin[eby]]
    eslot = np.empty(E, np.int64)
    eslot[eby] = ewin[eby] * S + epos

    rel_t = (np.asarray(inputs["last_update"]).astype(np.int64)[src]
             - np.asarray(inputs["t"]).astype(np.int64)).astype(np.float32)

    xg = np.zeros((NPAD, D), np.float32)
    xg[gslot] = x

    psrc_g = np.zeros(W * S, np.int32)
    pedge_g = np.zeros(W * S, np.int32)
    dstloc_g = -np.ones(W * S, np.float32)
    relt_g = np.zeros(W * S, np.float32)
    psrc_g[eslot] = gslot[src].astype(np.int32)
    pedge_g[eslot] = np.arange(E, dtype=np.int32)
    dstloc_g[eslot] = (gslot[dst] % NS).astype(np.float32)
    relt_g[eslot] = rel_t

    T_TOT = WPC * (S // TS)
    per_core = []
    for c in range(C):
        sl = slice(c * EPC, (c + 1) * EPC)
        # slot s -> (tile t = s // TS, lane p = s % TS); store [128, T_TOT]
        def tilize(a):
            return np.ascontiguousarray(a[sl].reshape(T_TOT, TS).T)
        per_core.append(dict(
            xp=np.ascontiguousarray(xg[c * WPC * NS:(c + 1) * WPC * NS]),
            psrcT=tilize(psrc_g),
            pedgeT=tilize(pedge_g),
            dstT=tilize(dstloc_g),
            relt=np.ascontiguousarray(relt_g[sl])[None, :],
        ))
    return xg, per_core, eslot


def fold_weights(inputs):
    f = {k: np.asarray(v, dtype=np.float32) if np.asarray(v).dtype.kind == "f"
         else np.asarray(v) for k, v in inputs.items()}
    out = {}
    aug = lambda w, b: np.ascontiguousarray(
        np.concatenate([w, b[None]], 0).astype(np.float32))
    out["kw_aug"] = aug(f["c1_kw"], f["c1_kb"])                      # [101,800]
    out["dvw_aug"] = aug(f["c1_vw"] - f["c1_kw"], f["c1_vb"] - f["c1_kb"])
    out["qw_aug"] = aug(f["c1_qw"], f["c1_qb"])
    out["sw_aug"] = aug(f["c1_sw"], f["c1_sb"])
    out["ew"] = np.ascontiguousarray(f["c1_ew"])                     # [200,800]
    w2 = np.concatenate([f["c2_qw"], f["c2_kw"], f["c2_vw"], f["c2_sw"]], 1)
    b2 = np.concatenate([f["c2_qb"], f["c2_kb"], f["c2_vb"], f["c2_sb"]])
    out["w2cat"] = aug(w2, b2)                                       # [801,400]
    out["c2_ew"] = np.ascontiguousarray(f["c2_ew"])                  # [200,100]
    out["A_s"] = np.ascontiguousarray(f["lp_src_w"] @ f["lp1_w"][:200])
    out["A_d"] = np.ascontiguousarray(f["lp_dst_w"] @ f["lp1_w"][200:])
    out["b1p"] = np.ascontiguousarray(
        f["lp1_b"] + f["lp_src_b"] @ f["lp1_w"][:200]
        + f["lp_dst_b"] @ f["lp1_w"][200:]).reshape(-1, 1)           # [800,1]
    out["lp2_w"] = np.ascontiguousarray(f["lp2_w"])                  # [800,200]
    out["lp2_b"] = f["lp2_b"].reshape(-1, 1)
    out["lp3_w"] = np.ascontiguousarray(f["lp3_w"])                  # [200,50]
    out["lp3_b"] = f["lp3_b"].reshape(-1, 1)
    out["lp4_w"] = np.ascontiguousarray(f["lp4_w"])                  # [50,2]
    out["lp4_b"] = f["lp4_b"].reshape(-1, 1)
    out["time_w"] = np.ascontiguousarray(f["time_w"])                # [1,100]
    out["tb_sin"] = np.ascontiguousarray(f["time_b"]).reshape(-1, 1)  # raw time_b
    return out


# ------------------------------ device program ------------------------------

def _chunks(total, step=128):
    return [(s, min(step, total - s)) for s in range(0, total, step)]


def build_program(W, S, NPAD, E_msg):
    """Build the SPMD Bass/Tile program (identical for all 8 cores)."""
    from contextlib import ExitStack
    import concourse.bass as bass
    import concourse.mybir as mybir
    import concourse.tile as tile
    import concourse.bacc as bacc
    from concourse.masks import make_identity

    dt = mybir.dt
    AF = mybir.ActivationFunctionType
    OP = mybir.AluOpType
    AX = mybir.AxisListType

    WPC = W // C
    TPW = S // TS
    NPC = WPC * NS
    EPC = WPC * S
    T_TOT = WPC * TPW

    nc = bacc.Bacc("TRN2", target_bir_lowering=False, debug=False,
                   num_devices=C)

    # ---- DRAM I/O ----
    di = lambda n, sh, d=dt.float32: nc.dram_tensor(n, sh, d, kind="ExternalInput").ap()
    xg_d = di("xg", [NPAD, D])
    xp_d = di("xp", [NPC, D])
    msg_d = di("msgin", [E_msg, D])
    relt_d = di("relt", [1, EPC])
    dstT_d = di("dstT", [TS, T_TOT])
    psrcT_d = di("psrcT", [TS, T_TOT], dt.int32)
    pedgeT_d = di("pedgeT", [TS, T_TOT], dt.int32)
    kw_d = di("kw_aug", [101, 800])
    dvw_d = di("dvw_aug", [101, 800])
    qw_d = di("qw_aug", [101, 800])
    sw_d = di("sw_aug", [101, 800])
    ew_d = di("ew", [200, 800])
    w2_d = di("w2cat", [801, 400])
    c2ew_d = di("c2_ew", [200, D])
    As_d = di("A_s", [D, 800])
    Ad_d = di("A_d", [D, 800])
    b1p_d = di("b1p", [800, 1])
    lp2w_d = di("lp2_w", [800, 200])
    lp2b_d = di("lp2_b", [200, 1])
    lp3w_d = di("lp3_w", [200, 50])
    lp3b_d = di("lp3_b", [50, 1])
    lp4w_d = di("lp4_w", [50, 2])
    lp4b_d = di("lp4_b", [2, 1])
    tw_d = di("time_w", [1, D])
    tbs_d = di("tb_sin", [D, 1])

    outT_d = nc.dram_tensor("outT", [2, EPC], dt.float32, kind="ExternalOutput").ap()

    # ---- internal DRAM (collectives need Shared outputs, plain internals ok) ----
    kvc_d = nc.dram_tensor("kv_contrib", [NPC, 200], dt.float32).ap()
    kvt_d = nc.dram_tensor("kv_table", [NPAD, 200], dt.float32,
                           addr_space="Shared").ap()
    h2c_d = nc.dram_tensor("h2_contrib", [NPC, D], dt.float32).ap()
    h2t_d = nc.dram_tensor("h2_table", [NPAD, D], dt.float32,
                           addr_space="Shared").ap()
    re_spill_d = nc.dram_tensor("re_spill", [D, EPC], dt.float32).ap()

    RG = [list(range(C))]

    with tile.TileContext(nc) as tc, ExitStack() as ctx:
        const = ctx.enter_context(tc.tile_pool(name="const", bufs=1))
        resid = ctx.enter_context(tc.tile_pool(name="resid", bufs=1))
        pwin = ctx.enter_context(tc.tile_pool(name="pwin", bufs=2))
        pt = ctx.enter_context(tc.tile_pool(name="pt", bufs=2))
        pbig = ctx.enter_context(tc.tile_pool(name="pbig", bufs=2))
        pe1 = ctx.enter_context(tc.tile_pool(name="pe1", bufs=1))
        ps_num = ctx.enter_context(tc.tile_pool(name="ps_num", bufs=1, space="PSUM"))
        ps_kj = ctx.enter_context(tc.tile_pool(name="ps_kj", bufs=1, space="PSUM"))
        ps_qv = ctx.enter_context(tc.tile_pool(name="ps_qv", bufs=1, space="PSUM"))
        ps_tr = ctx.enter_context(tc.tile_pool(name="ps_tr", bufs=2, space="PSUM"))

        f32 = dt.float32

        # ---- constants to SBUF ----
        ident = const.tile([128, 128], f32)
        make_identity(nc, ident[:])
        iota_i = const.tile([128, 128], dt.int32)
        nc.gpsimd.iota(iota_i[:], pattern=[[1, 128]], base=0, channel_multiplier=0)
        iota_f = const.tile([128, 128], f32)
        nc.vector.tensor_copy(iota_f[:], iota_i[:])
        ones_row = const.tile([1, 128], f32)
        nc.vector.memset(ones_row[:], 1.0)
        bias_zero = const.tile([128, 1], f32)
        nc.vector.memset(bias_zero[:], 0.0)
        bias_magic = const.tile([128, 1], f32)
        nc.vector.memset(bias_magic[:], 12582912.0)
        bias_nmagic = const.tile([128, 1], f32)
        nc.vector.memset(bias_nmagic[:], -12582912.0)
        bias_hpi = const.tile([128, 1], f32)
        nc.vector.memset(bias_hpi[:], float(np.float32(np.pi / 2.0)))

        def load_const(name, ap, shape, dtype=f32):
            t = const.tile(list(shape), dtype, name=name)
            nc.sync.dma_start(out=t[:], in_=ap)
            return t

        kw_s = load_const("kw_s", kw_d[:, :], [101, 800])
        dvw_s = load_const("dvw_s", dvw_d[:, :], [101, 800])
        qw_s = load_const("qw_s", qw_d[:, :], [101, 800])
        sw_s = load_const("sw_s", sw_d[:, :], [101, 800])
        ew0_s = load_const("ew0_s", ew_d[0:100, :], [100, 800])
        ew1_s = load_const("ew1_s", ew_d[100:200, :], [100, 800])
        c2ew0_s = load_const("c2ew0_s", c2ew_d[0:100, :], [100, D])
        c2ew1_s = load_const("c2ew1_s", c2ew_d[100:200, :], [100, D])
        As_s = load_const("As_s", As_d[:, :], [D, 800])
        Ad_s = load_const("Ad_s", Ad_d[:, :], [D, 800])
        lp3w_s = const.tile([128, 100], f32)  # chunk k at cols [50k:50k+50]
        nc.sync.dma_start(out=lp3w_s[0:128, 0:50], in_=lp3w_d[0:128, :])
        nc.sync.dma_start(out=lp3w_s[0:72, 50:100], in_=lp3w_d[128:200, :])
        lp4w_s = load_const("lp4w_s", lp4w_d[:, :], [50, 2])
        lp2b_s = const.tile([128, 2], f32)
        nc.sync.dma_start(out=lp2b_s[0:128, 0:1], in_=lp2b_d[0:128, :])
        nc.sync.dma_start(out=lp2b_s[0:72, 1:2], in_=lp2b_d[128:200, :])
        lp3b_s = load_const("lp3b_s", lp3b_d[:, :], [50, 1])
        lp4b_s = load_const("lp4b_s", lp4b_d[:, :], [2, 1])
        tw_s = load_const("tw_s", tw_d[:, :], [1, D])
        tbs_s = load_const("tbs_s", tbs_d[:, :], [D, 1])

        w2_ch = []
        for c_, (s_, n_) in enumerate(_chunks(800)):
            t = const.tile([n_, 400], f32, name=f"w2ch{c_}")
            nc.sync.dma_start(out=t[:], in_=w2_d[s_:s_ + n_, :])
            w2_ch.append((t, n_))
        b2_s = load_const("b2_s", w2_d[800:801, :], [1, 400])
        lp2_ch = []
        for c_, (s_, n_) in enumerate(_chunks(800)):
            t = const.tile([n_, 200], f32, name=f"lp2ch{c_}")
            nc.sync.dma_start(out=t[:], in_=lp2w_d[s_:s_ + n_, :])
            lp2_ch.append((t, n_))
        b1p_s = const.tile([128, 7], f32)
        for c_, (s_, n_) in enumerate(_chunks(800)):
            nc.sync.dma_start(out=b1p_s[0:n_, c_:c_ + 1], in_=b1p_d[s_:s_ + n_, :])

        # ---- per-core index tables ----
        dstT_s = const.tile([TS, T_TOT], f32)
        nc.sync.dma_start(out=dstT_s[:], in_=dstT_d[:, :])
        psrcT_s = const.tile([TS, T_TOT], dt.int32)
        nc.sync.dma_start(out=psrcT_s[:], in_=psrcT_d[:, :])
        pedgeT_s = const.tile([TS, T_TOT], dt.int32)
        nc.sync.dma_start(out=pedgeT_s[:], in_=pedgeT_d[:, :])

        # ---- residents (skip2 is spilled to DRAM to save SBUF) ----
        q2_all = resid.tile([NS, WPC * D], f32)
        h2_all = resid.tile([NS, WPC * D], f32)
        s2_d = nc.dram_tensor("s2_spill", [NPC, D], dt.float32).ap()

        def build_S(t):
            """one-hot S [128e,128n] and its transpose St for tile index t."""
            S_sb = pt.tile([TS, NS], f32, tag="S_sb")
            nc.vector.tensor_scalar(out=S_sb[:], in0=iota_f[:],
                                    scalar1=dstT_s[:, t:t + 1], scalar2=None,
                                    op0=OP.is_equal)
            st_ps = ps_tr.tile([NS, TS], f32, tag="tr")
            nc.tensor.transpose(out=st_ps[:], in_=S_sb[:], identity=ident[:])
            St_sb = pt.tile([NS, TS], f32, tag="St_sb")
            nc.scalar.copy(out=St_sb[:], in_=st_ps[:])
            return S_sb, St_sb

        # rel_enc = cos(rel_t*w + b): args reach +-3e6, but ScalarE Sin only
        # covers [-pi, pi]. Cody-Waite reduction with exact fp32 products
        # (6.25 and 17/512 have few significand bits, so k*c is exact),
        # one-sided wrap, and the +pi/2 cos->sin shift folded into the final
        # Sin bias (applied after reduction, where the argument is O(1)).
        MAGIC = 12582912.0                       # 1.5 * 2**23
        C_INV = float(np.float32(1.0 / (2.0 * np.pi)))
        C1 = 6.25
        C2 = float(np.float32(17.0 / 512.0))     # 0.033203125
        C3 = float(np.float32(2.0 * np.pi - 6.25 - 17.0 / 512.0))
        HALF_PI = float(np.float32(np.pi / 2.0))
        TWO_PI = float(np.float32(2.0 * np.pi))

        def rel_enc_compute(relt_w, j, re_sb):
            re_ps = ps_tr.tile([D, TS], f32, tag="tr")
            nc.tensor.matmul(out=re_ps[:], lhsT=tw_s[:],
                             rhs=relt_w[0:1, j * TS:(j + 1) * TS],
                             start=True, stop=True)
            X = pt.tile([D, TS], f32, tag="rrX")
            nc.scalar.activation(X[:], re_ps[:], AF.Identity, bias=tbs_s[:], scale=1.0)
            kk = pt.tile([D, TS], f32, tag="rrk")
            nc.scalar.activation(kk[:], X[:], AF.Identity, bias=bias_magic[0:D, :], scale=C_INV)
            nc.scalar.activation(kk[:], kk[:], AF.Identity, bias=bias_nmagic[0:D, :], scale=1.0)
            kc = pt.tile([D, TS], f32, tag="rrkc")
            nc.scalar.activation(kc[:], kk[:], AF.Identity, bias=bias_zero[0:D, :], scale=C1)
            nc.vector.tensor_tensor(out=X[:], in0=X[:], in1=kc[:], op=OP.subtract)
            nc.scalar.activation(kc[:], kk[:], AF.Identity, bias=bias_zero[0:D, :], scale=C2)
            nc.vector.tensor_tensor(out=X[:], in0=X[:], in1=kc[:], op=OP.subtract)
            nc.scalar.activation(kc[:], kk[:], AF.Identity, bias=bias_zero[0:D, :], scale=C3)
            nc.vector.tensor_tensor(out=X[:], in0=X[:], in1=kc[:], op=OP.subtract)
            nc.vector.tensor_scalar(out=kc[:], in0=X[:], scalar1=HALF_PI,
                                    scalar2=TWO_PI, op0=OP.is_gt, op1=OP.mult)
            nc.vector.tensor_tensor(out=X[:], in0=X[:], in1=kc[:], op=OP.subtract)
            nc.scalar.activation(re_sb[:], X[:], AF.Sin, bias=bias_hpi[0:D, :], scale=1.0)

        def attr_chunks(t, relt_w, j, phase_a):
            """edge_attr^T FM chunks (rel_enc [100,TS], msg^T [100,TS])."""
            re_sb = pt.tile([D, TS], f32, tag="re_sb")
            if phase_a:
                rel_enc_compute(relt_w, j, re_sb)
                nc.sync.dma_start(out=re_spill_d[:, t * TS:(t + 1) * TS],
                                  in_=re_sb[:])
            else:
                nc.sync.dma_start(out=re_sb[:],
                                  in_=re_spill_d[:, t * TS:(t + 1) * TS])
            mq = pt.tile([TS, D], f32, tag="mq")
            nc.gpsimd.indirect_dma_start(
                out=mq[:], out_offset=None, in_=msg_d[:, :],
                in_offset=bass.IndirectOffsetOnAxis(ap=pedgeT_s[:, t:t + 1], axis=0))
            mg_ps = ps_tr.tile([D, TS], f32, tag="tr")
            nc.tensor.transpose(out=mg_ps[:], in_=mq[:], identity=ident[:])
            mg_sb = pt.tile([D, TS], f32, tag="mg_sb")
            nc.scalar.copy(out=mg_sb[:], in_=mg_ps[:])
            return re_sb, mg_sb

        # =================== phase A + B (layer 1 + projections) ===========
        for w in range(WPC):
            relt_w = pwin.tile([1, S], f32, tag="relt_w")
            nc.sync.dma_start(out=relt_w[:], in_=relt_d[0:1, w * S:(w + 1) * S])
            xp_blk = pwin.tile([NS, D + 1], f32, tag="xp_blk")
            nc.sync.dma_start(out=xp_blk[:, 0:D], in_=xp_d[w * NS:(w + 1) * NS, :])
            nc.vector.memset(xp_blk[:, D:D + 1], 1.0)
            xpT_ps = ps_tr.tile([D + 1, NS], f32, tag="tr")
            nc.tensor.transpose(out=xpT_ps[:], in_=xp_blk[:], identity=ident[:])
            xpT_aug = pwin.tile([D + 1, NS], f32, tag="xpT_aug")
            nc.scalar.copy(out=xpT_aug[:], in_=xpT_ps[:])

            q_ps_a = ps_qv.tile([NS, 400], f32, tag="qv_a")
            q_ps_b = ps_qv.tile([NS, 400], f32, tag="qv_b")
            nc.tensor.matmul(out=q_ps_a[:], lhsT=xpT_aug[:], rhs=qw_s[:, 0:400],
                             start=True, stop=True)
            nc.tensor.matmul(out=q_ps_b[:], lhsT=xpT_aug[:], rhs=qw_s[:, 400:800],
                             start=True, stop=True)
            q_blk = pwin.tile([NS, 800], f32, tag="q_blk")
            nc.scalar.copy(out=q_blk[:, 0:400], in_=q_ps_a[:])
            nc.scalar.copy(out=q_blk[:, 400:800], in_=q_ps_b[:])

            num_a = ps_num.tile([NS, 400], f32, tag="num_a")
            num_b = ps_num.tile([NS, 408], f32, tag="num_b")

            for j in range(TPW):
                t = w * TPW + j
                e0 = w * S + j * TS
                S_sb, St_sb = build_S(t)

                # q_dst expansion
                qd_a = ps_qv.tile([TS, 400], f32, tag="qv_a")
                qd_b = ps_qv.tile([TS, 400], f32, tag="qv_b")
                nc.tensor.matmul(out=qd_a[:], lhsT=St_sb[:], rhs=q_blk[:, 0:400],
                                 start=True, stop=True)
                nc.tensor.matmul(out=qd_b[:], lhsT=St_sb[:], rhs=q_blk[:, 400:800],
                                 start=True, stop=True)

                # gather x[src], build xsT_aug (ones col appended pre-transpose)
                xs = pt.tile([TS, D + 1], f32, tag="xs")
                nc.gpsimd.indirect_dma_start(
                    out=xs[:, 0:D], out_offset=None, in_=xg_d[:, :],
                    in_offset=bass.IndirectOffsetOnAxis(ap=psrcT_s[:, t:t + 1], axis=0))
                nc.vector.memset(xs[:, D:D + 1], 1.0)
                xsT_ps = ps_tr.tile([D + 1, TS], f32, tag="tr")
                nc.tensor.transpose(out=xsT_ps[:], in_=xs[:], identity=ident[:])
                xsT_aug = pt.tile([D + 1, TS], f32, tag="xsT_aug")
                nc.scalar.copy(out=xsT_aug[:], in_=xsT_ps[:])

                re_sb, mg_sb = attr_chunks(t, relt_w, j, True)

                # kj = x@kw_aug + re@ew0 + mg@ew1   [128e, 800]
                kj_a = ps_kj.tile([TS, 400], f32, tag="kj_a")
                kj_b = ps_kj.tile([TS, 400], f32, tag="kj_b")
                for half, kp in ((0, kj_a), (1, kj_b)):
                    sl = slice(half * 400, half * 400 + 400)
                    nc.tensor.matmul(out=kp[:], lhsT=xsT_aug[:], rhs=kw_s[:, sl],
                                     start=True, stop=False)
                    nc.tensor.matmul(out=kp[:], lhsT=re_sb[:], rhs=ew0_s[:, sl],
                                     start=False, stop=False)
                    nc.tensor.matmul(out=kp[:], lhsT=mg_sb[:], rhs=ew1_s[:, sl],
                                     start=False, stop=True)

                # evict kj to SBUF (DVE may read only one PSUM operand)
                kj_sb = pbig.tile([TS, 800], f32, tag="kj_sb")
                nc.scalar.copy(out=kj_sb[:, 0:400], in_=kj_a[:])
                nc.scalar.copy(out=kj_sb[:, 400:800], in_=kj_b[:])

                # alpha = sum_d q_dst * kj   (per head)
                prod = pbig.tile([TS, 800], f32, tag="prod")
                nc.vector.tensor_tensor(out=prod[:, 0:400], in0=kj_sb[:, 0:400],
                                        in1=qd_a[:], op=OP.mult)
                nc.vector.tensor_tensor(out=prod[:, 400:800], in0=kj_sb[:, 400:800],
                                        in1=qd_b[:], op=OP.mult)
                alpha = pt.tile([TS, 8], f32, tag="alpha")
                nc.vector.reduce_sum(out=alpha[:],
                                     in_=prod[:].rearrange("p (h d) -> p h d", d=D),
                                     axis=AX.X)
                ex = pt.tile([TS, 8], f32, tag="ex")
                nc.scalar.activation(ex[:], alpha[:], AF.Exp, bias=bias_zero[0:TS, :], scale=0.1)

                # vj = kj + x@dvw_aug ; w = ex*vj | ex
                vj_a = ps_qv.tile([TS, 400], f32, tag="qv_a")
                vj_b = ps_qv.tile([TS, 400], f32, tag="qv_b")
                nc.tensor.matmul(out=vj_a[:], lhsT=xsT_aug[:], rhs=dvw_s[:, 0:400],
                                 start=True, stop=True)
                nc.tensor.matmul(out=vj_b[:], lhsT=xsT_aug[:], rhs=dvw_s[:, 400:800],
                                 start=True, stop=True)
                w_sb = pbig.tile([TS, 808], f32, tag="w_sb")
                nc.vector.tensor_tensor(out=w_sb[:, 0:400], in0=kj_sb[:, 0:400],
                                        in1=vj_a[:], op=OP.add)
                nc.vector.tensor_tensor(out=w_sb[:, 400:800], in0=kj_sb[:, 400:800],
                                        in1=vj_b[:], op=OP.add)
                for h in range(8):
                    nc.vector.tensor_scalar_mul(
                        out=w_sb[:, h * D:(h + 1) * D],
                        in0=w_sb[:, h * D:(h + 1) * D], scalar1=ex[:, h:h + 1])
                nc.vector.tensor_copy(out=w_sb[:, 800:808], in_=ex[:])

                # scatter-add into per-window numerator/denominator
                nc.tensor.matmul(out=num_a[:], lhsT=S_sb[:], rhs=w_sb[:, 0:400],
                                 start=(j == 0), stop=(j == TPW - 1))
                nc.tensor.matmul(out=num_b[:], lhsT=S_sb[:], rhs=w_sb[:, 400:808],
                                 start=(j == 0), stop=(j == TPW - 1))

            # ---- window eviction: h1 = relu(num/den + x@sw_aug) ----
            den = pwin.tile([NS, 8], f32, tag="den")
            nc.vector.tensor_scalar_add(out=den[:], in0=num_b[:, 400:408],
                                        scalar1=1e-16)
            rcp = pwin.tile([NS, 8], f32, tag="rcp")
            nc.vector.reciprocal(out=rcp[:], in_=den[:])
            agg = pwin.tile([NS, 800], f32, tag="agg")
            for h in range(8):
                src_ps = num_a if h < 4 else num_b
                off = h * D if h < 4 else (h - 4) * D
                nc.vector.tensor_scalar_mul(out=agg[:, h * D:(h + 1) * D],
                                            in0=src_ps[:, off:off + D],
                                            scalar1=rcp[:, h:h + 1])
            skip_a = ps_kj.tile([NS, 400], f32, tag="kj_a")
            skip_b = ps_kj.tile([NS, 400], f32, tag="kj_b")
            nc.tensor.matmul(out=skip_a[:], lhsT=xpT_aug[:], rhs=sw_s[:, 0:400],
                             start=True, stop=True)
            nc.tensor.matmul(out=skip_b[:], lhsT=xpT_aug[:], rhs=sw_s[:, 400:800],
                             start=True, stop=True)
            h1 = pwin.tile([NS, 800], f32, tag="h1")
            nc.vector.tensor_tensor(out=h1[:, 0:400], in0=agg[:, 0:400],
                                    in1=skip_a[:], op=OP.add)
            nc.vector.tensor_tensor(out=h1[:, 400:800], in0=agg[:, 400:800],
                                    in1=skip_b[:], op=OP.add)
            nc.scalar.activation(h1[:], h1[:], AF.Relu, bias=bias_zero[0:NS, :], scale=1.0)

            # h1^T chunks -> phase B matmul y2 = [q2|k2|v2|s2]
            h1T = pwin.tile([128, 7 * 128], f32, tag="h1T")
            for c_, (s_, n_) in enumerate(_chunks(800)):
                tr_ps = ps_tr.tile([128, NS], f32, tag="tr")
                nc.tensor.transpose(out=tr_ps[0:n_, :], in_=h1[:, s_:s_ + n_],
                                    identity=ident[:])
                nc.scalar.copy(out=h1T[0:n_, c_ * 128:c_ * 128 + NS],
                               in_=tr_ps[0:n_, :])
            y2 = ps_tr.tile([NS, 400], f32, tag="tr")
            for c_, (w2t, n_) in enumerate(w2_ch):
                nc.tensor.matmul(out=y2[:], lhsT=h1T[0:n_, c_ * 128:c_ * 128 + NS],
                                 rhs=w2t[:], start=(c_ == 0), stop=False)
            nc.tensor.matmul(out=y2[:], lhsT=ones_row[:], rhs=b2_s[:],
                             start=False, stop=True)
            nc.scalar.copy(out=q2_all[:, w * D:(w + 1) * D], in_=y2[:, 0:D])
            s2_sb = pwin.tile([NS, D], f32, tag="s2_sb")
            nc.scalar.copy(out=s2_sb[:], in_=y2[:, 300:400])
            nc.sync.dma_start(out=s2_d[w * NS:(w + 1) * NS, :], in_=s2_sb[:])
            kv_sb = pwin.tile([NS, 200], f32, tag="kv_sb")
            nc.scalar.copy(out=kv_sb[:], in_=y2[:, 100:300])
            nc.sync.dma_start(out=kvc_d[w * NS:(w + 1) * NS, :], in_=kv_sb[:])

        # =================== allgather k2|v2 ===============================
        nc.gpsimd.collective_compute(
            "AllGather", mybir.AluOpType.bypass, replica_groups=RG,
            ins=[kvc_d.opt()], outs=[kvt_d.opt()])

        # =================== phase C (layer 2) =============================
        for w in range(WPC):
            relt_w = pwin.tile([1, S], f32, tag="relt_w")
            nc.sync.dma_start(out=relt_w[:], in_=relt_d[0:1, w * S:(w + 1) * S])
            num2 = ps_num.tile([NS, 104], f32, tag="num_a")
            for j in range(TPW):
                t = w * TPW + j
                S_sb, St_sb = build_S(t)
                re_sb, mg_sb = attr_chunks(t, relt_w, j, False)
                e2_ps = ps_kj.tile([TS, D], f32, tag="kj_a")
                nc.tensor.matmul(out=e2_ps[:], lhsT=re_sb[:], rhs=c2ew0_s[:],
                                 start=True, stop=False)
                nc.tensor.matmul(out=e2_ps[:], lhsT=mg_sb[:], rhs=c2ew1_s[:],
                                 start=False, stop=True)
                q2d_ps = ps_qv.tile([TS, D], f32, tag="qv_a")
                nc.tensor.matmul(out=q2d_ps[:], lhsT=St_sb[:],
                                 rhs=q2_all[:, w * D:(w + 1) * D],
                                 start=True, stop=True)
                kvg = pt.tile([TS, 200], f32, tag="kvg")
                nc.gpsimd.indirect_dma_start(
                    out=kvg[:], out_offset=None, in_=kvt_d[:, :],
                    in_offset=bass.IndirectOffsetOnAxis(ap=psrcT_s[:, t:t + 1], axis=0))
                kj2 = pt.tile([TS, D], f32, tag="kj2")
                nc.vector.tensor_tensor(out=kj2[:], in0=kvg[:, 0:D], in1=e2_ps[:],
                                        op=OP.add)
                prod2 = pt.tile([TS, D], f32, tag="prod2")
                nc.vector.tensor_tensor(out=prod2[:], in0=kj2[:], in1=q2d_ps[:],
                                        op=OP.mult)
                alpha2 = pt.tile([TS, 1], f32, tag="alpha2")
                nc.vector.reduce_sum(out=alpha2[:], in_=prod2[:], axis=AX.X)
                ex2 = pt.tile([TS, 1], f32, tag="ex2")
                nc.scalar.activation(ex2[:], alpha2[:], AF.Exp, bias=bias_zero[0:TS, :], scale=0.1)
                w2_sb = pt.tile([TS, 101], f32, tag="w2_sb")
                nc.vector.tensor_tensor(out=w2_sb[:, 0:D], in0=kvg[:, D:200],
                                        in1=e2_ps[:], op=OP.add)
                nc.vector.tensor_scalar_mul(out=w2_sb[:, 0:D], in0=w2_sb[:, 0:D],
                                            scalar1=ex2[:])
                nc.vector.tensor_copy(out=w2_sb[:, D:101], in_=ex2[:])
                nc.tensor.matmul(out=num2[:, 0:101], lhsT=S_sb[:], rhs=w2_sb[:],
                                 start=(j == 0), stop=(j == TPW - 1))
            den2 = pwin.tile([NS, 1], f32, tag="den2")
            nc.vector.tensor_scalar_add(out=den2[:], in0=num2[:, D:D + 1],
                                        scalar1=1e-16)
            rcp2 = pwin.tile([NS, 1], f32, tag="rcp2")
            nc.vector.reciprocal(out=rcp2[:], in_=den2[:])
            s2_sb = pwin.tile([NS, D], f32, tag="s2_sb")
            nc.sync.dma_start(out=s2_sb[:], in_=s2_d[w * NS:(w + 1) * NS, :])
            h2w = h2_all[:, w * D:(w + 1) * D]
            nc.vector.tensor_scalar_mul(out=h2w, in0=num2[:, 0:D], scalar1=rcp2[:])
            nc.vector.tensor_tensor(out=h2w, in0=h2w, in1=s2_sb[:], op=OP.add)
            nc.scalar.activation(h2w, h2w, AF.Relu, bias=bias_zero[0:NS, :], scale=1.0)
            h2_sb = pwin.tile([NS, D], f32, tag="h2_sb")
            nc.vector.tensor_copy(out=h2_sb[:], in_=h2w)
            nc.sync.dma_start(out=h2c_d[w * NS:(w + 1) * NS, :], in_=h2_sb[:])

        # =================== allgather h2 ==================================
        nc.gpsimd.collective_compute(
            "AllGather", mybir.AluOpType.bypass, replica_groups=RG,
            ins=[h2c_d.opt()], outs=[h2t_d.opt()])

        # =================== phase E (LinkPredictor) =======================
        SE = S  # process a whole window's edges at once, feature-major
        for w in range(WPC):
            St_w = pt.tile([NS, SE], f32, tag="St_w")
            hsT = pt.tile([D, SE], f32, tag="hsT")
            for j in range(TPW):
                t = w * TPW + j
                S_sb = pt.tile([TS, NS], f32, tag="S_sb")
                nc.vector.tensor_scalar(out=S_sb[:], in0=iota_f[:],
                                        scalar1=dstT_s[:, t:t + 1], scalar2=None,
                                        op0=OP.is_equal)
                st_ps = ps_tr.tile([NS, TS], f32, tag="tr")
                nc.tensor.transpose(out=st_ps[:], in_=S_sb[:], identity=ident[:])
                nc.scalar.copy(out=St_w[:, j * TS:(j + 1) * TS], in_=st_ps[:])
                hs = pt.tile([TS, D], f32, tag="hs")
                nc.gpsimd.indirect_dma_start(
                    out=hs[:], out_offset=None, in_=h2t_d[:, :],
                    in_offset=bass.IndirectOffsetOnAxis(ap=psrcT_s[:, t:t + 1], axis=0))
                hs_ps = ps_tr.tile([D, TS], f32, tag="tr")
                nc.tensor.transpose(out=hs_ps[:], in_=hs[:], identity=ident[:])
                nc.scalar.copy(out=hsT[:, j * TS:(j + 1) * TS], in_=hs_ps[:])
            hdT_ps = ps_qv.tile([D, SE], f32, tag="qv_a")
            nc.tensor.matmul(out=hdT_ps[:], lhsT=h2_all[:, w * D:(w + 1) * D],
                             rhs=St_w[:], start=True, stop=True)
            hdT = pt.tile([D, SE], f32, tag="hdT")
            nc.scalar.copy(out=hdT[:], in_=hdT_ps[:])

            t1 = pe1.tile([128, 7 * SE], f32, tag="t1")
            for c_, (s_, n_) in enumerate(_chunks(800)):
                t1_ps = ps_num.tile([128, SE], f32, tag="num_a" if c_ % 2 == 0 else "num_b")
                nc.tensor.matmul(out=t1_ps[0:n_, :], lhsT=As_s[:, s_:s_ + n_],
                                 rhs=hsT[:], start=True, stop=False)
                nc.tensor.matmul(out=t1_ps[0:n_, :], lhsT=Ad_s[:, s_:s_ + n_],
                                 rhs=hdT[:], start=False, stop=True)
                nc.scalar.activation(t1[0:n_, c_ * SE:(c_ + 1) * SE], t1_ps[0:n_, :],
                                     AF.Tanh, bias=b1p_s[0:n_, c_:c_ + 1], scale=1.0)
            t2 = pe1.tile([128, 2 * SE], f32, tag="t2")
            for m, (ms, mn) in enumerate(_chunks(200)):
                t2_ps = ps_kj.tile([128, SE], f32, tag="kj_a" if m == 0 else "kj_b")
                for c_, (lp2t, n_) in enumerate(lp2_ch):
                    nc.tensor.matmul(out=t2_ps[0:mn, :],
                                     lhsT=lp2t[:, ms:ms + mn],
                                     rhs=t1[0:n_, c_ * SE:(c_ + 1) * SE],
                                     start=(c_ == 0), stop=(c_ == 6))
                nc.scalar.activation(t2[0:mn, m * SE:(m + 1) * SE], t2_ps[0:mn, :],
                                     AF.Tanh, bias=lp2b_s[0:mn, m:m + 1], scale=1.0)
            t3_ps = ps_qv.tile([50, SE], f32, tag="qv_b")
            nc.tensor.matmul(out=t3_ps[:], lhsT=lp3w_s[0:128, 0:50],
                             rhs=t2[0:128, 0:SE], start=True, stop=False)
            nc.tensor.matmul(out=t3_ps[:], lhsT=lp3w_s[0:72, 50:100],
                             rhs=t2[0:72, SE:2 * SE], start=False, stop=True)
            t3 = pwin.tile([50, SE], f32, tag="t3")
            nc.scalar.activation(t3[:], t3_ps[:], AF.Tanh, bias=lp3b_s[:], scale=1.0)
            out_ps = ps_tr.tile([2, SE], f32, tag="tr")
            nc.tensor.matmul(out=out_ps[:], lhsT=lp4w_s[:], rhs=t3[:],
                             start=True, stop=True)
            out_sb = pwin.tile([2, SE], f32, tag="out_sb")
            nc.scalar.activation(out_sb[:], out_ps[:], AF.Identity,
                                 bias=lp4b_s[:], scale=1.0)
            nc.sync.dma_start(out=outT_d[:, w * S:(w + 1) * S], in_=out_sb[:])

    nc.compile()
    return nc


# ------------------------------- entry point -------------------------------

_CACHE = {}


def _get_program(W, S, NPAD, E_msg):
    key = (W, S, NPAD, E_msg)
    if key not in _CACHE:
        _CACHE[key] = build_program(W, S, NPAD, E_msg)
    return _CACHE[key]


def kernel(**inputs):
    W, S = 400, 384
    dst = np.asarray(inputs["edge_index"][1])
    # adapt S upward in the (improbable) case the packing overflows 384
    # (checked cheaply via the same snake assignment inside preprocess)
    for S_try in (384, 512, 640, 768):
        try:
            xg, per_core, eslot = preprocess(inputs, W, S_try)
            S = S_try
            break
        except AssertionError:
            continue
    else:
        raise RuntimeError("window packing failed")

    fw = fold_weights(inputs)
    NPAD = W * NS
    E = dst.shape[0]
    msg = np.ascontiguousarray(np.asarray(inputs["msg"], dtype=np.float32))

    nc = _get_program(W, S, NPAD, msg.shape[0])

    shared = dict(xg=xg, msgin=msg, kw_aug=fw["kw_aug"], dvw_aug=fw["dvw_aug"],
                  qw_aug=fw["qw_aug"], sw_aug=fw["sw_aug"], ew=fw["ew"],
                  w2cat=fw["w2cat"], c2_ew=fw["c2_ew"], A_s=fw["A_s"],
                  A_d=fw["A_d"], b1p=fw["b1p"], lp2_w=fw["lp2_w"],
                  lp2_b=fw["lp2_b"], lp3_w=fw["lp3_w"], lp3_b=fw["lp3_b"],
                  lp4_w=fw["lp4_w"], lp4_b=fw["lp4_b"], time_w=fw["time_w"],
                  tb_sin=fw["tb_sin"])
    in_maps = []
    for c in range(C):
        pc = per_core[c]
        in_maps.append(dict(shared, xp=pc["xp"], relt=pc["relt"],
                            dstT=pc["dstT"], psrcT=pc["psrcT"],
                            pedgeT=pc["pedgeT"]))

    from concourse import bass_utils
    res = bass_utils.run_bass_kernel_spmd(nc, in_maps, core_ids=list(range(C)))

    EPC = (W // C) * S
    outT = np.stack([res.results[c]["outT"] for c in range(C)])  # [C,2,EPC]
    flat = outT.transpose(0, 2, 1).reshape(W * S, 2)             # [W*S, 2]
    return np.ascontiguousarray(flat[eslot]).astype(np.float32)

